# revision 21
# baseline (speedup 1.0000x reference)
"""AssociativeMemory (scatter_memory) Trainium2 kernel.

Math (exactly equivalent to the sequential reference):
  idx[b,j]  = validate(vectors[b,j]) = min(v, 256)   (inputs are ints in [0,300))
  count[i,j]= #{b : idx[b,j]==i}
  out[i,j]  = min(rel[i,j] + count[i,j], 1023)        (+1-with-clamp is order-free)
  recognized[b] = all_j out[idx[b,j], j] > 0
              == (min(rel) + min_j count[idx[b,j], j]) > 0 for rel >= 0
                 (every touched cell holds min(rel_cell + c, 1023) >= rel_min + c_min)

Design: the n columns are sharded across 8 cores. Each core's table shard is
TRANSPOSED on the host to [n_loc, 257] so each original column is one
contiguous ~1KB row. The device then:
  - streams the shard through SBUF in tiles of 128 rows x (7*257) (plain DMA),
  - builds the sparse count tile with gpsimd local_scatter (per-partition
    scatter along the free dim; duplicate indices pre-merged via pair counts),
  - adds counts (DVE) and clamps via two scalar-engine relu passes
    (min(x,1023) = 1023 - relu(1023 - x)),
  - streams tiles back out; the host transposes the result back.
The per-(b,j) pair counts and the recognize reductions are computed in a
column-major [128, 489, 8] layout prepared host-side.
"""

import sys

for p in ("/opt/trn_rl_repo",):
    if p not in sys.path:
        sys.path.insert(0, p)

import numpy as np

import concourse.bacc as bacc
import concourse.bass as bass
import concourse.mybir as mybir
import concourse.tile as tile
from concourse import bass_utils, library_config

F32 = mybir.dt.float32
I32 = mybir.dt.int32
I16 = mybir.dt.int16

M1 = 257          # relation rows (m+1)
B = 8             # number of cue vectors
ABS_MAX = 1023.0
N_FULL = 500_000
N_CORES = 8
N_LOC = N_FULL // N_CORES        # 62500 columns per core
C_PER_PART = 489                 # columns per partition (128*489 = 62592)
N_PAD = 128 * C_PER_PART         # 62592
G_TILES = 7                      # table rows (=orig columns) per batch per partition
ROW_PAD = M1 + 1                 # 258, local_scatter num_elems must be even


def build_nc(c_per_part=C_PER_PART, m1=M1, b=B, g_tiles=G_TILES):
    n_pad = 128 * c_per_part
    bc = b * c_per_part
    row_pad = m1 + 1

    nc = bacc.Bacc(
        "TRN2",
        target_bir_lowering=False,
        debug=False,
        enable_asserts=False,
        num_devices=1,
    )
    relT = nc.dram_tensor("relation", [n_pad, m1], F32, kind="ExternalInput").ap()
    vec = nc.dram_tensor("vectors", [128, bc], I32, kind="ExternalInput").ap()
    outT = nc.dram_tensor("out", [n_pad, m1], F32, kind="ExternalOutput").ap()
    recb = nc.dram_tensor("recb", [1, b], F32, kind="ExternalOutput").ap()
    rtmin = nc.dram_tensor("rtmin", [1, 1], F32, kind="ExternalOutput").ap()

    n_batches = (c_per_part + g_tiles - 1) // g_tiles
    AOT = mybir.AluOpType

    with tile.TileContext(nc) as tc:
        with (
            tc.tile_pool(name="prep", bufs=1) as prep,
            tc.tile_pool(name="loop", bufs=4) as loop,
        ):
            vec_sb = prep.tile([128, bc], I32, tag="vec")
            idx16 = prep.tile([128, bc], I16, tag="idx16")
            cnt = prep.tile([128, bc], I16, tag="cnt")
            dup = prep.tile([128, bc], I16, tag="dup")
            lsidx = prep.tile([128, bc], I16, tag="lsidx")
            etmp = prep.tile([128, c_per_part], I16, tag="etmp")
            ramp = prep.tile([128, b * g_tiles], I16, tag="ramp")
            cminf = prep.tile([128, b], F32, tag="cminf")
            rmin = prep.tile([128, 1], F32, tag="rmin")
            rec_sb = prep.tile([1, b], F32, tag="recsb")
            rt_sb = prep.tile([1, 1], F32, tag="rtsb")
            clamp_bias = prep.tile([128, 1], F32, tag="clampb")
            nc.vector.memset(clamp_bias[:], ABS_MAX)

            # ramp for batching local_scatter: tile t in batch -> +t*row_pad
            nc.gpsimd.iota(
                ramp[:], pattern=[[row_pad, g_tiles], [0, b]], base=0,
                channel_multiplier=0,
            )

            nc.sync.dma_start(out=vec_sb[:], in_=vec[:, :])

            # validate: idx = min(v, 256), as int16
            nc.vector.tensor_scalar(
                out=idx16[:], in0=vec_sb[:], scalar1=m1 - 1, scalar2=None,
                op0=AOT.min,
            )

            # pair counts + duplicate flags, layout [128, c, b] (b innermost)
            idx3 = idx16[:].rearrange("p (c b) -> p c b", b=b)
            cnt3 = cnt[:].rearrange("p (c b) -> p c b", b=b)
            dup3 = dup[:].rearrange("p (c b) -> p c b", b=b)
            nc.vector.memset(cnt[:], 1)
            nc.vector.memset(dup[:], 0)
            for a in range(b):
                for bb in range(a + 1, b):
                    nc.vector.tensor_tensor(
                        out=etmp[:], in0=idx3[:, :, a], in1=idx3[:, :, bb],
                        op=AOT.is_equal,
                    )
                    nc.vector.tensor_add(
                        out=cnt3[:, :, a], in0=cnt3[:, :, a], in1=etmp[:]
                    )
                    nc.vector.tensor_add(
                        out=cnt3[:, :, bb], in0=cnt3[:, :, bb], in1=etmp[:]
                    )
                    nc.vector.tensor_add(
                        out=dup3[:, :, bb], in0=dup3[:, :, bb], in1=etmp[:]
                    )

            # local_scatter forbids duplicates: idx -> negative for non-first dups
            nc.vector.tensor_scalar(
                out=dup[:], in0=dup[:], scalar1=1, scalar2=None, op0=AOT.min
            )
            # offset must stay negative after the batch ramp adds up to
            # row_pad*(g_tiles-1): use -8192
            nc.vector.scalar_tensor_tensor(
                out=lsidx[:], in0=dup[:], scalar=-8192, in1=idx16[:],
                op0=AOT.mult, op1=AOT.add,
            )

            # recognize part 1: min_j count[b] (exact), via strided view
            cnt_t = cnt[:].rearrange("p (c b) -> p b c", b=b)
            nc.vector.tensor_reduce(
                out=cminf[:], in_=cnt_t, axis=mybir.AxisListType.X, op=AOT.min
            )
            nc.vector.memset(rmin[:], 3.0e38)

            # main streamed loop over row batches
            relT3 = relT.rearrange("(p c) r -> p c r", p=128)
            outT3 = outT.rearrange("(p c) r -> p c r", p=128)
            for t0 in range(n_batches):
                nt = min(g_tiles, c_per_part - t0 * g_tiles)
                ne = row_pad * nt
                ni = b * nt
                csl = slice(t0 * g_tiles, t0 * g_tiles + nt)
                fsl = slice(t0 * g_tiles * b, t0 * g_tiles * b + ni)

                idxb = loop.tile([128, b * g_tiles], I16, tag="idxb")
                cntt = loop.tile([128, row_pad * g_tiles], I16, tag="cntt")
                tbuf = loop.tile([128, m1 * g_tiles], F32, tag="tbuf")
                rb = loop.tile([128, 1], F32, tag="rb")

                nc.vector.tensor_add(
                    out=idxb[:, :ni], in0=lsidx[:, fsl], in1=ramp[:, :ni]
                )
                nc.gpsimd.local_scatter(
                    out_ap=cntt[:, :ne], data_ap=cnt[:, fsl],
                    idxs_ap=idxb[:, :ni],
                    channels=128, num_elems=ne, num_idxs=ni,
                )
                nc.sync.dma_start(out=tbuf[:, : m1 * nt], in_=relT3[:, csl, :])

                # rel-min for recognize (before the add)
                nc.vector.tensor_reduce(
                    out=rb[:], in_=tbuf[:, : m1 * nt],
                    axis=mybir.AxisListType.X, op=AOT.min,
                )
                nc.vector.tensor_tensor(
                    out=rmin[:], in0=rmin[:], in1=rb[:], op=AOT.min
                )

                # out = T + cnt (mixed i16+f32), then clamp via 2 ACT passes:
                # y = relu(1023 - x); out = 1023 - y
                tb3 = tbuf[:, : m1 * nt].rearrange("p (c r) -> p c r", r=m1)
                ct3 = cntt[:, :ne].rearrange("p (c r) -> p c r", r=row_pad)
                nc.vector.tensor_tensor(
                    out=tb3, in0=tb3, in1=ct3[:, :, :m1], op=AOT.add
                )
                nc.scalar.activation(
                    out=tbuf[:, : m1 * nt], in_=tbuf[:, : m1 * nt],
                    func=mybir.ActivationFunctionType.Relu,
                    bias=clamp_bias[:], scale=-1.0,
                )
                nc.scalar.activation(
                    out=tbuf[:, : m1 * nt], in_=tbuf[:, : m1 * nt],
                    func=mybir.ActivationFunctionType.Copy,
                    bias=ABS_MAX, scale=-1.0,
                )
                nc.sync.dma_start(out=outT3[:, csl, :], in_=tbuf[:, : m1 * nt])

            # recognize part 2: cross-partition mins via -max(-x)
            nc.vector.tensor_scalar(
                out=cminf[:], in0=cminf[:], scalar1=-1.0, scalar2=None,
                op0=AOT.mult,
            )
            nc.vector.tensor_scalar(
                out=rmin[:], in0=rmin[:], scalar1=-1.0, scalar2=None,
                op0=AOT.mult,
            )
            nc.gpsimd.tensor_reduce(
                out=rec_sb[:], in_=cminf[:], axis=mybir.AxisListType.C,
                op=AOT.max,
            )
            nc.gpsimd.tensor_reduce(
                out=rt_sb[:], in_=rmin[:], axis=mybir.AxisListType.C,
                op=AOT.max,
            )
            nc.sync.dma_start(out=recb[:, :], in_=rec_sb[:])
            nc.sync.dma_start(out=rtmin[:, :], in_=rt_sb[:])

    nc.compile()
    return nc


_NC_CACHE = {}


def _get_nc():
    if "nc" not in _NC_CACHE:
        _NC_CACHE["nc"] = build_nc()
    return _NC_CACHE["nc"]


def make_in_maps(relation, vectors):
    relation = np.asarray(relation, dtype=np.float32)
    vectors = np.asarray(vectors, dtype=np.int32)
    in_maps = []
    for c in range(N_CORES):
        sl = slice(c * N_LOC, (c + 1) * N_LOC)
        rel_t = np.zeros((N_PAD, M1), np.float32)
        rel_t[:N_LOC] = relation[:, sl].T
        vec_l = np.zeros((B, N_PAD), np.int32)
        vec_l[:, :N_LOC] = vectors[:, sl]
        vec_img = np.ascontiguousarray(
            vec_l.reshape(B, 128, C_PER_PART).transpose(1, 2, 0)
        ).reshape(128, C_PER_PART * B)
        in_maps.append({"relation": rel_t, "vectors": vec_img})
    return in_maps


def assemble(results):
    rel_out = np.empty((M1, N_FULL), np.float32)
    for c in range(N_CORES):
        sl = slice(c * N_LOC, (c + 1) * N_LOC)
        rel_out[:, sl] = results[c]["out"][:N_LOC].T
    cmin = -np.stack([results[c]["recb"] for c in range(N_CORES)]).max(axis=0)
    relmin = -max(float(results[c]["rtmin"].max()) for c in range(N_CORES))
    recognized = ((relmin + cmin) > 0).reshape(B)
    return rel_out, recognized


def kernel(relation, vectors):
    in_maps = make_in_maps(relation, vectors)
    nc = _get_nc()
    res = bass_utils.run_bass_kernel_spmd(nc, in_maps, core_ids=list(range(N_CORES)))
    return assemble(res.results)


# revision 22
# speedup vs baseline: 1.2314x; 1.2314x over previous
"""AssociativeMemory (scatter_memory) Trainium2 kernel.

Math (exactly equivalent to the sequential reference):
  idx[b,j]  = validate(vectors[b,j]) = min(v, 256)   (inputs are ints in [0,300))
  count[i,j]= #{b : idx[b,j]==i}
  out[i,j]  = min(rel[i,j] + count[i,j], 1023)        (+1-with-clamp is order-free)
  recognized[b] = all_j out[idx[b,j], j] > 0
              == (min(rel) + min_j count[idx[b,j], j]) > 0 for rel >= 0
                 (every touched cell holds min(rel_cell + c, 1023) >= rel_min + c_min)

Design: columns sharded across 8 cores; each core's shard is TRANSPOSED on the
host to [n_loc, 257] so one original column = one contiguous ~1KB row. Device:
  - streams the shard through SBUF in [128 rows x 7*257] tiles, cast to int16
    during the DMA (values are integers <= 1031, exact in i16; DVE i16 ops run
    ~2x f32),
  - builds the sparse count tile with gpsimd local_scatter (per-partition
    free-dim scatter; duplicates pre-merged via pair counts),
  - int16 add on DVE; clamp min(x,1023)=1023-relu(1023-x) as 2 scalar-engine
    passes; DMA out casts back to f32,
  - recognize: rel-min reduce (DVE) + per-b count-min, cross-partition via
    gpsimd max of negated values.
Pair counts run in a b-major [128, 8, 489] layout (contiguous slices), then
one strided copy each rearranges counts/indices to the c-major layout that
local_scatter's contiguity rules need.
"""

import sys

for p in ("/opt/trn_rl_repo",):
    if p not in sys.path:
        sys.path.insert(0, p)

import numpy as np

import concourse.bacc as bacc
import concourse.bass as bass
import concourse.mybir as mybir
import concourse.tile as tile
from concourse import bass_utils, library_config

F32 = mybir.dt.float32
I32 = mybir.dt.int32
I16 = mybir.dt.int16

M1 = 257          # relation rows (m+1)
B = 8             # number of cue vectors
ABS_MAX = 1023.0
N_FULL = 500_000
N_CORES = 8
N_LOC = N_FULL // N_CORES        # 62500 columns per core
C_PER_PART = 489                 # columns per partition (128*489 = 62592)
N_PAD = 128 * C_PER_PART         # 62592
G_TILES = 7                      # table rows (=orig columns) per batch per partition
ROW_PAD = M1 + 1                 # 258, local_scatter num_elems must be even


def build_nc(c_per_part=C_PER_PART, m1=M1, b=B, g_tiles=G_TILES):
    n_pad = 128 * c_per_part
    bc = b * c_per_part
    row_pad = m1 + 1

    nc = bacc.Bacc(
        "TRN2",
        target_bir_lowering=False,
        debug=False,
        enable_asserts=False,
        num_devices=1,
    )
    relT = nc.dram_tensor("relation", [n_pad, m1], F32, kind="ExternalInput").ap()
    vec = nc.dram_tensor("vectors", [128, bc], I32, kind="ExternalInput").ap()
    outT = nc.dram_tensor("out", [n_pad, m1], F32, kind="ExternalOutput").ap()
    recb = nc.dram_tensor("recb", [1, b], F32, kind="ExternalOutput").ap()
    rtmin = nc.dram_tensor("rtmin", [1, 1], F32, kind="ExternalOutput").ap()

    n_batches = (c_per_part + g_tiles - 1) // g_tiles
    AOT = mybir.AluOpType

    with tile.TileContext(nc) as tc:
        with (
            tc.tile_pool(name="prep", bufs=1) as prep,
            tc.tile_pool(name="loop", bufs=4) as loop,
        ):
            vec_sb = prep.tile([128, bc], I32, tag="vec")
            idx16 = prep.tile([128, bc], I16, tag="idx16")   # b-major [8, 489]
            cnt = prep.tile([128, bc], I16, tag="cnt")       # b-major
            dup = prep.tile([128, bc], I16, tag="dup")       # b-major
            cnt_c = prep.tile([128, bc], I16, tag="cntc")    # c-major [489, 8]
            lsidx_c = prep.tile([128, bc], I16, tag="lsidxc")
            etmp = prep.tile([128, c_per_part], I16, tag="etmp")
            ramp = prep.tile([128, b * g_tiles], I16, tag="ramp")
            cminf = prep.tile([128, b], F32, tag="cminf")
            rmin = prep.tile([128, 1], I16, tag="rmin")
            rminf = prep.tile([128, 1], F32, tag="rminf")
            rec_sb = prep.tile([1, b], F32, tag="recsb")
            rt_sb = prep.tile([1, 1], F32, tag="rtsb")
            clamp_bias = prep.tile([128, 1], F32, tag="clampb")
            nc.vector.memset(clamp_bias[:], ABS_MAX)

            # ramp for batching local_scatter: tile t in batch -> +t*row_pad
            nc.gpsimd.iota(
                ramp[:], pattern=[[row_pad, g_tiles], [0, b]], base=0,
                channel_multiplier=0,
            )

            nc.sync.dma_start(out=vec_sb[:], in_=vec[:, :])

            # validate: idx = min(v, 256), as int16 (b-major layout)
            nc.vector.tensor_scalar(
                out=idx16[:], in0=vec_sb[:], scalar1=m1 - 1, scalar2=None,
                op0=AOT.min,
            )

            # pair counts + duplicate flags on contiguous b-major slices
            idx3 = idx16[:].rearrange("p (b c) -> p b c", b=b)
            cnt3 = cnt[:].rearrange("p (b c) -> p b c", b=b)
            dup3 = dup[:].rearrange("p (b c) -> p b c", b=b)
            nc.vector.memset(cnt[:], 1)
            nc.vector.memset(dup[:], 0)
            for a in range(b):
                for bb in range(a + 1, b):
                    nc.vector.tensor_tensor(
                        out=etmp[:], in0=idx3[:, a, :], in1=idx3[:, bb, :],
                        op=AOT.is_equal,
                    )
                    nc.vector.tensor_add(
                        out=cnt3[:, a, :], in0=cnt3[:, a, :], in1=etmp[:]
                    )
                    nc.vector.tensor_add(
                        out=cnt3[:, bb, :], in0=cnt3[:, bb, :], in1=etmp[:]
                    )
                    nc.vector.tensor_add(
                        out=dup3[:, bb, :], in0=dup3[:, bb, :], in1=etmp[:]
                    )

            # recognize part 1: min_j count[b] (contiguous reduce in b-major)
            nc.vector.tensor_reduce(
                out=cminf[:], in_=cnt3, axis=mybir.AxisListType.X, op=AOT.min
            )

            # local_scatter forbids duplicates: idx -> negative for non-first
            # dups (offset stays negative after the +258*t ramp: use -8192);
            # write results straight into the c-major tiles (strided store)
            nc.vector.tensor_scalar(
                out=dup[:], in0=dup[:], scalar1=1, scalar2=None, op0=AOT.min
            )
            cntc3 = cnt_c[:].rearrange("p (c b) -> p b c", b=b)
            lsic3 = lsidx_c[:].rearrange("p (c b) -> p b c", b=b)
            for a in range(b):
                nc.vector.scalar_tensor_tensor(
                    out=lsic3[:, a, :], in0=dup3[:, a, :], scalar=-8192,
                    in1=idx3[:, a, :], op0=AOT.mult, op1=AOT.add,
                )
                nc.vector.tensor_copy(out=cntc3[:, a, :], in_=cnt3[:, a, :])

            nc.vector.memset(rmin[:], 32767)

            # main streamed loop over row batches (int16 pipeline, DMA casts)
            relT3 = relT.rearrange("(p c) r -> p c r", p=128)
            outT3 = outT.rearrange("(p c) r -> p c r", p=128)
            for t0 in range(n_batches):
                nt = min(g_tiles, c_per_part - t0 * g_tiles)
                ne = row_pad * nt
                ni = b * nt
                csl = slice(t0 * g_tiles, t0 * g_tiles + nt)
                fsl = slice(t0 * g_tiles * b, t0 * g_tiles * b + ni)

                idxb = loop.tile([128, b * g_tiles], I16, tag="idxb")
                cntt = loop.tile([128, row_pad * g_tiles], I16, tag="cntt")
                tbuf = loop.tile([128, m1 * g_tiles], I16, tag="tbuf")
                rb = loop.tile([128, 1], I16, tag="rb")

                nc.vector.tensor_add(
                    out=idxb[:, :ni], in0=lsidx_c[:, fsl], in1=ramp[:, :ni]
                )
                nc.gpsimd.local_scatter(
                    out_ap=cntt[:, :ne], data_ap=cnt_c[:, fsl],
                    idxs_ap=idxb[:, :ni],
                    channels=128, num_elems=ne, num_idxs=ni,
                )
                # DMA casts f32 -> i16 (SWDGE)
                nc.gpsimd.dma_start(out=tbuf[:, : m1 * nt], in_=relT3[:, csl, :])

                # rel-min for recognize (before the add)
                nc.vector.tensor_reduce(
                    out=rb[:], in_=tbuf[:, : m1 * nt],
                    axis=mybir.AxisListType.X, op=AOT.min,
                )
                nc.vector.tensor_tensor(
                    out=rmin[:], in0=rmin[:], in1=rb[:], op=AOT.min
                )

                # out = T + cnt (i16), then clamp via 2 ACT passes:
                # y = relu(1023 - x); out = 1023 - y
                tb3 = tbuf[:, : m1 * nt].rearrange("p (c r) -> p c r", r=m1)
                ct3 = cntt[:, :ne].rearrange("p (c r) -> p c r", r=row_pad)
                nc.vector.tensor_tensor(
                    out=tb3, in0=tb3, in1=ct3[:, :, :m1], op=AOT.add
                )
                nc.scalar.activation(
                    out=tbuf[:, : m1 * nt], in_=tbuf[:, : m1 * nt],
                    func=mybir.ActivationFunctionType.Relu,
                    bias=clamp_bias[:], scale=-1.0,
                )
                nc.scalar.activation(
                    out=tbuf[:, : m1 * nt], in_=tbuf[:, : m1 * nt],
                    func=mybir.ActivationFunctionType.Copy,
                    bias=ABS_MAX, scale=-1.0,
                )
                # DMA casts i16 -> f32 (SWDGE)
                nc.gpsimd.dma_start(out=outT3[:, csl, :], in_=tbuf[:, : m1 * nt])

            # recognize part 2: cross-partition mins via -max(-x)
            nc.vector.tensor_scalar(
                out=cminf[:], in0=cminf[:], scalar1=-1.0, scalar2=None,
                op0=AOT.mult,
            )
            nc.vector.tensor_scalar(
                out=rminf[:], in0=rmin[:], scalar1=-1.0, scalar2=None,
                op0=AOT.mult,
            )
            nc.gpsimd.tensor_reduce(
                out=rec_sb[:], in_=cminf[:], axis=mybir.AxisListType.C,
                op=AOT.max,
            )
            nc.gpsimd.tensor_reduce(
                out=rt_sb[:], in_=rminf[:], axis=mybir.AxisListType.C,
                op=AOT.max,
            )
            nc.sync.dma_start(out=recb[:, :], in_=rec_sb[:])
            nc.sync.dma_start(out=rtmin[:, :], in_=rt_sb[:])

    nc.compile()
    return nc


_NC_CACHE = {}


def _get_nc():
    if "nc" not in _NC_CACHE:
        _NC_CACHE["nc"] = build_nc()
    return _NC_CACHE["nc"]


def make_in_maps(relation, vectors):
    relation = np.asarray(relation, dtype=np.float32)
    vectors = np.asarray(vectors, dtype=np.int32)
    in_maps = []
    for c in range(N_CORES):
        sl = slice(c * N_LOC, (c + 1) * N_LOC)
        rel_t = np.zeros((N_PAD, M1), np.float32)
        rel_t[:N_LOC] = relation[:, sl].T
        vec_l = np.zeros((B, N_PAD), np.int32)
        vec_l[:, :N_LOC] = vectors[:, sl]
        # b-major per partition: [128][b][489]
        vec_img = np.ascontiguousarray(
            vec_l.reshape(B, 128, C_PER_PART).transpose(1, 0, 2)
        ).reshape(128, B * C_PER_PART)
        in_maps.append({"relation": rel_t, "vectors": vec_img})
    return in_maps


def assemble(results):
    rel_out = np.empty((M1, N_FULL), np.float32)
    for c in range(N_CORES):
        sl = slice(c * N_LOC, (c + 1) * N_LOC)
        rel_out[:, sl] = results[c]["out"][:N_LOC].T
    cmin = -np.stack([results[c]["recb"] for c in range(N_CORES)]).max(axis=0)
    relmin = -max(float(results[c]["rtmin"].max()) for c in range(N_CORES))
    recognized = ((relmin + cmin) > 0).reshape(B)
    return rel_out, recognized


def kernel(relation, vectors):
    in_maps = make_in_maps(relation, vectors)
    nc = _get_nc()
    res = bass_utils.run_bass_kernel_spmd(nc, in_maps, core_ids=list(range(N_CORES)))
    return assemble(res.results)


# revision 28
# speedup vs baseline: 1.4301x; 1.1613x over previous
"""AssociativeMemory (scatter_memory) Trainium2 kernel.

Math (exactly equivalent to the sequential reference):
  idx[b,j]  = validate(vectors[b,j]) = min(v, 256)   (inputs are ints in [0,300))
  count[i,j]= #{b : idx[b,j]==i}
  out[i,j]  = min(rel[i,j] + count[i,j], 1023)        (+1-with-clamp is order-free)
  recognized[b] = all_j out[idx[b,j], j] > 0
              == (min(rel) + min_j count[idx[b,j], j]) > 0 for rel >= 0
                 (every touched cell holds min(rel_cell + c, 1023) >= rel_min + c_min)

Design: columns sharded across 8 cores; each core's shard is TRANSPOSED on the
host to [n_loc, 257] so one original column = one contiguous ~1KB row. Device:
  - streams the shard through SBUF in [128 rows x 7*257] tiles, cast to int16
    during the DMA (values are integers <= 1031, exact in i16; DVE i16 ops run
    ~2x f32),
  - builds the sparse count tile with gpsimd local_scatter (per-partition
    free-dim scatter; duplicates pre-merged via pair counts),
  - int16 add on DVE; clamp min(x,1023)=1023-relu(1023-x) as 2 scalar-engine
    passes; DMA out casts back to f32,
  - recognize: rel-min reduce (DVE) + per-b count-min, cross-partition via
    gpsimd max of negated values.
Pair counts run in a b-major [128, 8, 489] layout (contiguous slices), then
one strided copy each rearranges counts/indices to the c-major layout that
local_scatter's contiguity rules need.
"""

import sys

for p in ("/opt/trn_rl_repo",):
    if p not in sys.path:
        sys.path.insert(0, p)

import numpy as np

import concourse.bacc as bacc
import concourse.bass as bass
import concourse.mybir as mybir
import concourse.tile as tile
from concourse import bass_utils, library_config

F32 = mybir.dt.float32
I32 = mybir.dt.int32
I16 = mybir.dt.int16

M1 = 257          # relation rows (m+1)
B = 8             # number of cue vectors
ABS_MAX = 1023.0
N_FULL = 500_000
N_CORES = 8
N_LOC = N_FULL // N_CORES        # 62500 columns per core
C_PER_PART = 489                 # columns per partition (128*489 = 62592)
N_PAD = 128 * C_PER_PART         # 62592
G_TILES = 7                      # table rows (=orig columns) per batch per partition
ROW_PAD = M1 + 1                 # 258, local_scatter num_elems must be even


def build_nc(c_per_part=C_PER_PART, m1=M1, b=B, g_tiles=G_TILES):
    n_pad = 128 * c_per_part
    bc = b * c_per_part
    row_pad = m1 + 1

    nc = bacc.Bacc(
        "TRN2",
        target_bir_lowering=False,
        debug=False,
        enable_asserts=False,
        num_devices=1,
    )
    relT = nc.dram_tensor("relation", [n_pad, m1], F32, kind="ExternalInput").ap()
    vec = nc.dram_tensor("vectors", [128, bc], I32, kind="ExternalInput").ap()
    outT = nc.dram_tensor("out", [n_pad, m1], F32, kind="ExternalOutput").ap()
    recb = nc.dram_tensor("recb", [1, b], F32, kind="ExternalOutput").ap()
    negm = nc.dram_tensor("negm", [1, 1], F32, kind="ExternalOutput").ap()

    n_batches = (c_per_part + g_tiles - 1) // g_tiles
    AOT = mybir.AluOpType

    with tile.TileContext(nc) as tc:
        with (
            tc.tile_pool(name="prep", bufs=1) as prep,
            tc.tile_pool(name="loop", bufs=4) as loop,
        ):
            vec_sb = prep.tile([128, bc], I32, tag="vec")
            idx16 = prep.tile([128, bc], I16, tag="idx16")   # b-major [8, 489]
            cnt = prep.tile([128, bc], I16, tag="cnt")       # b-major
            dup = prep.tile([128, bc], I16, tag="dup")       # b-major
            cnt_c = prep.tile([128, bc], I16, tag="cntc")    # c-major [489, 8]
            lsidx_c = prep.tile([128, bc], I16, tag="lsidxc")
            etmp = prep.tile([128, c_per_part], I16, tag="etmp")
            ramp = prep.tile([128, b * g_tiles], I16, tag="ramp")
            cminf = prep.tile([128, b], F32, tag="cminf")
            negacc = prep.tile([128, n_batches], F32, tag="negacc")
            negsum = prep.tile([128, 1], F32, tag="negsum")
            rec_sb = prep.tile([1, b], F32, tag="recsb")
            ng_sb = prep.tile([1, 1], F32, tag="ngsb")

            # ramp for batching local_scatter: tile t in batch -> +t*row_pad
            nc.gpsimd.iota(
                ramp[:], pattern=[[row_pad, g_tiles], [0, b]], base=0,
                channel_multiplier=0,
            )

            nc.sync.dma_start(out=vec_sb[:], in_=vec[:, :])

            # validate: idx = min(v, 256), as int16 (b-major layout)
            nc.vector.tensor_scalar(
                out=idx16[:], in0=vec_sb[:], scalar1=m1 - 1, scalar2=None,
                op0=AOT.min,
            )

            # pair counts + duplicate flags on contiguous b-major slices
            idx3 = idx16[:].rearrange("p (b c) -> p b c", b=b)
            cnt3 = cnt[:].rearrange("p (b c) -> p b c", b=b)
            dup3 = dup[:].rearrange("p (b c) -> p b c", b=b)
            nc.vector.memset(cnt[:], 1)
            nc.vector.memset(dup[:], 0)
            for a in range(b):
                for bb in range(a + 1, b):
                    nc.vector.tensor_tensor(
                        out=etmp[:], in0=idx3[:, a, :], in1=idx3[:, bb, :],
                        op=AOT.is_equal,
                    )
                    nc.vector.tensor_add(
                        out=cnt3[:, a, :], in0=cnt3[:, a, :], in1=etmp[:]
                    )
                    nc.vector.tensor_add(
                        out=cnt3[:, bb, :], in0=cnt3[:, bb, :], in1=etmp[:]
                    )
                    nc.vector.tensor_add(
                        out=dup3[:, bb, :], in0=dup3[:, bb, :], in1=etmp[:]
                    )

            # recognize part 1: min_j count[b] (contiguous reduce in b-major)
            nc.vector.tensor_reduce(
                out=cminf[:], in_=cnt3, axis=mybir.AxisListType.X, op=AOT.min
            )

            # local_scatter forbids duplicates: idx -> negative for non-first
            # dups (offset stays negative after the +258*t ramp: use -8192);
            # write results straight into the c-major tiles (strided store)
            nc.vector.tensor_scalar(
                out=dup[:], in0=dup[:], scalar1=1, scalar2=None, op0=AOT.min
            )
            cntc3 = cnt_c[:].rearrange("p (c b) -> p b c", b=b)
            lsic3 = lsidx_c[:].rearrange("p (c b) -> p b c", b=b)
            for a in range(b):
                nc.vector.scalar_tensor_tensor(
                    out=lsic3[:, a, :], in0=dup3[:, a, :], scalar=-8192,
                    in1=idx3[:, a, :], op0=AOT.mult, op1=AOT.add,
                )
                nc.vector.tensor_copy(out=cntc3[:, a, :], in_=cnt3[:, a, :])

            # main streamed loop over row batches (int16 pipeline, DMA casts)
            relT3 = relT.rearrange("(p c) r -> p c r", p=128)
            outT3 = outT.rearrange("(p c) r -> p c r", p=128)
            for t0 in range(n_batches):
                nt = min(g_tiles, c_per_part - t0 * g_tiles)
                ne = row_pad * nt
                ni = b * nt
                csl = slice(t0 * g_tiles, t0 * g_tiles + nt)
                fsl = slice(t0 * g_tiles * b, t0 * g_tiles * b + ni)

                idxb = loop.tile([128, b * g_tiles], I16, tag="idxb")
                cntt = loop.tile([128, row_pad * g_tiles], I16, tag="cntt")
                tbuf = loop.tile([128, m1 * g_tiles], I16, tag="tbuf")
                ascr = loop.tile([128, m1 * g_tiles], I16, tag="ascr")

                nc.vector.tensor_add(
                    out=idxb[:, :ni], in0=lsidx_c[:, fsl], in1=ramp[:, :ni]
                )
                nc.gpsimd.local_scatter(
                    out_ap=cntt[:, :ne], data_ap=cnt_c[:, fsl],
                    idxs_ap=idxb[:, :ni],
                    channels=128, num_elems=ne, num_idxs=ni,
                )
                # DMA casts f32 -> i16 (SWDGE)
                nc.gpsimd.dma_start(out=tbuf[:, : m1 * nt], in_=relT3[:, csl, :])

                # negativity certificate for recognize (scalar engine):
                # accumulate sum(relu(-T)) per partition; zero iff rel >= 0
                nc.scalar.activation(
                    out=ascr[:, : m1 * nt], in_=tbuf[:, : m1 * nt],
                    func=mybir.ActivationFunctionType.Relu,
                    bias=0.0, scale=-1.0,
                    accum_out=negacc[:, t0 : t0 + 1],
                )

                # out = min(T + cnt, 1023), all int16 on DVE
                tb3 = tbuf[:, : m1 * nt].rearrange("p (c r) -> p c r", r=m1)
                ct3 = cntt[:, :ne].rearrange("p (c r) -> p c r", r=row_pad)
                nc.vector.tensor_tensor(
                    out=tb3, in0=tb3, in1=ct3[:, :, :m1], op=AOT.add
                )
                nc.vector.tensor_scalar(
                    out=tbuf[:, : m1 * nt], in0=tbuf[:, : m1 * nt],
                    scalar1=int(ABS_MAX), scalar2=None, op0=AOT.min,
                )
                # DMA casts i16 -> f32 (SWDGE)
                nc.gpsimd.dma_start(out=outT3[:, csl, :], in_=tbuf[:, : m1 * nt])

            # recognize part 2: cross-partition reduce
            nc.vector.tensor_scalar(
                out=cminf[:], in0=cminf[:], scalar1=-1.0, scalar2=None,
                op0=AOT.mult,
            )
            nc.vector.tensor_reduce(
                out=negsum[:], in_=negacc[:], axis=mybir.AxisListType.X,
                op=AOT.add,
            )
            nc.gpsimd.tensor_reduce(
                out=rec_sb[:], in_=cminf[:], axis=mybir.AxisListType.C,
                op=AOT.max,
            )
            nc.gpsimd.tensor_reduce(
                out=ng_sb[:], in_=negsum[:], axis=mybir.AxisListType.C,
                op=AOT.add,
            )
            nc.sync.dma_start(out=recb[:, :], in_=rec_sb[:])
            nc.sync.dma_start(out=negm[:, :], in_=ng_sb[:])

    nc.compile()
    return nc


_NC_CACHE = {}


def _get_nc():
    if "nc" not in _NC_CACHE:
        _NC_CACHE["nc"] = build_nc()
    return _NC_CACHE["nc"]


def make_in_maps(relation, vectors):
    relation = np.asarray(relation, dtype=np.float32)
    vectors = np.asarray(vectors, dtype=np.int32)
    in_maps = []
    for c in range(N_CORES):
        sl = slice(c * N_LOC, (c + 1) * N_LOC)
        rel_t = np.zeros((N_PAD, M1), np.float32)
        rel_t[:N_LOC] = relation[:, sl].T
        vec_l = np.zeros((B, N_PAD), np.int32)
        vec_l[:, :N_LOC] = vectors[:, sl]
        # b-major per partition: [128][b][489]
        vec_img = np.ascontiguousarray(
            vec_l.reshape(B, 128, C_PER_PART).transpose(1, 0, 2)
        ).reshape(128, B * C_PER_PART)
        in_maps.append({"relation": rel_t, "vectors": vec_img})
    return in_maps


def assemble(results):
    rel_out = np.empty((M1, N_FULL), np.float32)
    for c in range(N_CORES):
        sl = slice(c * N_LOC, (c + 1) * N_LOC)
        rel_out[:, sl] = results[c]["out"][:N_LOC].T
    cmin = -np.stack([results[c]["recb"] for c in range(N_CORES)]).max(axis=0)
    negmass = sum(float(results[c]["negm"].ravel()[0]) for c in range(N_CORES))
    # no negative cells (negmass==0) -> every touched cell >= 0 + count >= 1
    recognized = ((negmass == 0.0) & (cmin >= 1)).reshape(B)
    return rel_out, recognized


def kernel(relation, vectors):
    in_maps = make_in_maps(relation, vectors)
    nc = _get_nc()
    res = bass_utils.run_bass_kernel_spmd(nc, in_maps, core_ids=list(range(N_CORES)))
    return assemble(res.results)
